# revision 39
# baseline (speedup 1.0000x reference)
"""Trainium2 Bass kernel for AdvancedKANLayer.

Math (per reference):
  xn    = LayerNorm(x) * ln_w + ln_b           (eps=1e-5)
  base  = silu(xn) @ base_weight.T             [B,S,O]
  t     = tanh(xn)
  basis = cos(pi*k*t), k=1..8
  spl   = einsum('bsig,oig->bso', basis, spline_weight)
  out   = base + spl
Strategy: data-parallel over batch (8 cores, one batch entry each, no
collectives).  Per core the whole thing is one K=18432 GEMM:
  out[o, t] = sum_k W_all[k, o] * panel[k, t]
where panel rows are [silu(xn); cos(1*pi*t); ...; cos(8*pi*t)] per
I-chunk, generated on-chip.  cos(k*pi*t) is built from
c1 = cos(pi*t) = 1 - 2*sin(pi*t/2)^2 via Chebyshev product
identities on the VectorEngine (ScalarE Sin is only valid on [-pi,pi]).

Mixed precision (rel-err budget 2e-2, measured ~1.7e-2): rows
{silu, cos7, cos8} run as fp8-e4m3 and are packed two-per-step into
MatmulPerfMode.DoubleRow matmuls (HW-verified: a DoubleRow pair-step
contracting 256 k costs the same ~216 ns as one bf16 128-k step, i.e.
2x throughput); rows cos1..cos6 stay bf16.  Per I-chunk that is
6 bf16 steps + 1.5 DoubleRow steps = 7.5 step-equivalents instead of
9 -> ~1.2x kernel speedup.  fp8 rows are paired per 2-ic block:
D1=(silu_e, c7_e), D2=(c8_e, silu_o), D3=(c7_o, c8_o), so all pair
tiles fill in generation order.  Weights are pre-transposed/pre-tiled
on the host (bf16 + fp8 planes); f32 PSUM accumulation throughout.

Perf notes (measured on HW): the N=512 matmul stream floor is ~216
ns/MM (1 col/cycle @2.4GHz) and LDWEIGHTS hides fully as long as the
weight DMAs stay ahead.  To that end: output DMAs issue on the ACT
HWDGE queue so the SP queue only carries weight/x DMAs; weight DMAs
move one 2-ic block per transfer (12 bf16 steps + 3 DR steps as two
DMAs) with deep pools; 5 o-tiles race the panel generation; the next
chunk's first six I-chunks are generated inside the tail of the
current mm sweep so the PE crosses chunk boundaries without idling;
240 warmup matmuls keep the HAM clock-gate open during the initial
LayerNorm; I-chunk transposes are emitted in pairs (8 back-to-back)
to halve PE mode switches; and a post-schedule pass (_optimize_sems)
strips unreferenced semaphore increments.
"""

import math
import sys
import types

try:  # some images lack antenv.axon_hooks, which bass_utils imports
    import antenv.axon_hooks  # noqa: F401
except Exception:
    try:
        import antenv
        _hooks = {}
        _m = types.ModuleType("antenv.axon_hooks")
        _m.set_axon_ntff_profile_hook = lambda h: _hooks.__setitem__("h", h)
        _m.get_axon_ntff_profile_hook = lambda: _hooks.get("h")
        sys.modules["antenv.axon_hooks"] = _m
        antenv.axon_hooks = _m
    except Exception:
        pass

import numpy as np
import ml_dtypes

import concourse.bass as bass
import concourse.mybir as mybir
import concourse.tile as tile
from concourse import bacc
from concourse import masks
from concourse.bass import ds, ts
from concourse.bass_utils import run_bass_kernel_spmd

F32 = mybir.dt.float32
BF16 = mybir.dt.bfloat16
FP8 = mybir.dt.float8e4
AF = mybir.ActivationFunctionType
ALU = mybir.AluOpType
PM = mybir.MatmulPerfMode

EPS = 1e-5

# geometry (full problem, per core)
B = 8
T = 2048          # tokens per core (= S, one batch entry per core)
I = 2048          # input dim
O = 2048          # output dim
G = 8             # cos harmonics
TCH = 512         # token chunk (matmul N)
NCH = T // TCH    # 4
NIC = I // 128    # 16 I-chunks
NBLK = NIC // 2   # 8 2-ic blocks
NBF = 6           # bf16 rows per ic (cos1..cos6)
NDR = 3           # DoubleRow pair-steps per regular 2-ic block
NDRX = 4          # max DR pair-steps (blocks 0-1 carry a 4th: c6 pair)
FP6_ICS = 4       # ics 0..3 run cos6 in fp8 too (quarter-row err spend)
NOT = O // 128    # 16 o-tiles
# per-chunk step counts
NSTEP_BF = NIC * NBF        # 96
NSTEP_DR = NBLK * NDR       # 24
NSTEP = NSTEP_BF + NSTEP_DR  # 120 PE matmul instructions per (ot, chunk)


def build_nc(nT=T, nI=I, nO=O, tch=TCH):
    nch = nT // tch
    nic = nI // 128
    nblk = nic // 2
    n_ot = nO // 128
    ntt = tch // 128          # token-tiles per chunk

    n_race = min(5, n_ot - 1) if n_ot > 1 else 1

    nc = bacc.Bacc("TRN2", target_bir_lowering=False, debug=False)
    x_ext = nc.declare_dram_parameter("x", [nT, nI], F32, isOutput=False)
    lnw_ext = nc.declare_dram_parameter("lnw", [nI], F32, isOutput=False)
    lnb_ext = nc.declare_dram_parameter("lnb", [nI], F32, isOutput=False)
    wtb_ext = nc.declare_dram_parameter(
        "wtb", [n_ot, nblk, 128, 2 * NBF, 128], BF16, isOutput=False)
    wtf_ext = nc.declare_dram_parameter(
        "wtf", [n_ot, nblk, 128, NDRX, 2, 128], FP8, isOutput=False)
    out_ext = nc.declare_dram_parameter("out", [nO, nT], F32, isOutput=True)

    with tile.TileContext(nc) as tc:
        with (
            tc.tile_pool(name="consts", bufs=1) as consts,
            tc.tile_pool(name="xp", bufs=4) as xpool,
            tc.tile_pool(name="statp", bufs=2) as statp,
            tc.tile_pool(name="genp", bufs=1) as genp,
            tc.tile_pool(name="ladp", bufs=1) as ladp,
            tc.tile_pool(name="panelp", bufs=1) as panelp,
            tc.tile_pool(name="wpb", bufs=6) as wpb,
            tc.tile_pool(name="wpf", bufs=6) as wpf,
            tc.tile_pool(name="stgp", bufs=2) as stgp,
            tc.tile_pool(name="tpps", bufs=3, space="PSUM") as tpps,
            tc.tile_pool(name="mmps", bufs=5, space="PSUM") as mmps,
        ):
            identity = consts.tile([128, 128], F32)
            masks.make_identity(nc, identity[:])
            lnw_sb = consts.tile([128, nic], F32)
            nc.sync.dma_start(lnw_sb[:], lnw_ext.rearrange("(f p) -> p f", p=128))
            lnb_sb = consts.tile([128, nic], F32)
            nc.sync.dma_start(lnb_sb[:], lnb_ext.rearrange("(f p) -> p f", p=128))
            eps_sb = consts.tile([128, 1], F32)
            nc.vector.memset(eps_sb[:], EPS)
            zb = consts.tile([128, 128], BF16)
            nc.vector.memset(zb[:], 0.0)

            # PE warmup: keep HAM busy while the first chunk's LN runs so
            # the first real matmuls start at full clock.
            wps = mmps.tile([128, tch], F32, tag="ps", name="warm_ps")
            for _ in range(320):
                nc.tensor.matmul(wps[:, 0:128], zb[:], zb[:])

            state = {}
            tpm = {}

            def preamble(c):
                """x DMA + LN stats + in-place normalize for chunk c.
                Stats/normalize are per token-tile so the first tile is
                ready after one x DMA, not four."""
                xnts = []
                for j in range(ntt):
                    xt = xpool.tile([128, nI], F32, tag="xt")
                    # x loads ride the ACT HWDGE queue (outputs only,
                    # mostly idle) so they never stall the SP weight queue
                    nc.scalar.dma_start(xt[:], x_ext[ds((c * ntt + j) * 128, 128), :])
                    bn6 = statp.tile([128, 4, 6], F32, tag="bn6")
                    for q in range(4):
                        nc.vector.bn_stats(
                            bn6[:, q, :], xt[:, ds(q * (nI // 4), nI // 4)]
                        )
                    stats = statp.tile([128, 2], F32, tag="stats")
                    nc.vector.bn_aggr(stats[:], bn6[:])
                    std = statp.tile([128, 1], F32, tag="std")
                    nc.scalar.activation(
                        std[:], stats[:, 1:2], AF.Sqrt, bias=eps_sb[:]
                    )
                    istd = statp.tile([128, 1], F32, tag="istd")
                    nc.vector.reciprocal(istd[:], std[:])
                    nmi = statp.tile([128, 1], F32, tag="nmi")
                    nc.vector.scalar_tensor_tensor(
                        nmi[:], stats[:, 0:1], -1.0, istd[:], ALU.mult, ALU.mult
                    )
                    # normalize in place: xn = (x - mu) * istd
                    nc.scalar.activation(
                        xt[:], xt[:], AF.Identity, bias=nmi[:], scale=istd[:],
                    )
                    xnts.append(xt)
                state[c] = xnts

            def transpose_ic(c, ic):
                """PE-transpose I-chunk ic of chunk c into a PSUM tile."""
                xnts = state[c]
                tp = tpps.tile([128, tch], F32, tag="tp", name=f"tp_{c}_{ic}")
                for j in range(ntt):
                    nc.tensor.transpose(
                        tp[:, ts(j, 128)], xnts[j][:, ts(ic, 128)], identity[:]
                    )
                tpm[(c, ic)] = tp
                return tp

            pre_ptiles = {}

            def gen_ic(c, ic, pbt, pft):
                """Transpose + tanh/silu + cheb ladder for I-chunk ic of
                chunk c.  bf16 rows cos1..cos6 fill pbt[ic*6 .. ic*6+5];
                fp8 rows (silu, cos7, cos8) fill their pair-tile slots in
                pft (3 pair tiles per 2-ic block)."""
                tp = tpm.pop((c, ic), None)
                if tp is None:
                    tp = transpose_ic(c, ic)
                lw = lnw_sb[:, ic : ic + 1]
                lb = lnb_sb[:, ic : ic + 1]
                j = ic // 2
                odd = ic % 2

                def pb(m):
                    # bf16 panel tile for cos_m (m=1..6)
                    s = ic * NBF + (m - 1)
                    t_ = panelp.tile(
                        [128, tch], BF16, tag=f"pb{s:03d}", name=f"pb_{c}_{s:03d}"
                    )
                    pbt[s] = t_
                    return t_

                def pfs(which):
                    # fp8 pair-tile slot for silu/cos7/cos8 (+cos6 on
                    # ics < FP6_ICS) of this ic
                    if not odd:
                        d, slot = {"silu": (0, 0), "c7": (0, 1), "c8": (1, 0),
                                   "c6": (3, 0)}[which]
                    else:
                        d, slot = {"silu": (1, 1), "c7": (2, 0), "c8": (2, 1),
                                   "c6": (3, 1)}[which]
                    di = j * NDRX + d
                    t_ = pft.get(di)
                    if t_ is None:
                        t_ = panelp.tile(
                            [128, 2, tch], FP8, tag=f"pf{di:02d}",
                            name=f"pf_{c}_{di:02d}"
                        )
                        pft[di] = t_
                    return t_[:, slot, :]

                th = genp.tile([128, tch], F32, tag="th")
                nc.scalar.activation(th[:], tp[:], AF.Tanh, bias=lb, scale=lw)

                nc.scalar.activation(pfs("silu"), tp[:], AF.Silu, bias=lb, scale=lw)
                sh = genp.tile([128, tch], F32, tag="sh")
                nc.scalar.activation(sh[:], th[:], AF.Sin, scale=math.pi / 2)

                def lad(tag):
                    return ladp.tile(
                        [128, tch], F32, tag=tag, name=f"lad_{tag}_{c}_{ic}"
                    )

                def stt(out, a, s, b):
                    nc.vector.scalar_tensor_tensor(
                        out[:], a[:], s, b[:], ALU.mult, ALU.mult
                    )

                # c1 = 1 - 2*sh^2
                u = lad("u")
                stt(u, sh, -2.0, sh)
                c1 = lad("c1")
                nc.vector.tensor_scalar_add(c1[:], u[:], 1.0)
                # squares on ScalarE to offload DVE
                sq1 = lad("sq")
                nc.scalar.square(sq1[:], c1[:])
                c2 = lad("c2")
                nc.vector.tensor_scalar(c2[:], sq1[:], 2.0, -1.0, ALU.mult, ALU.add)
                # c3 = 2*c1*c2 - c1
                u3 = lad("u")
                stt(u3, c2, 2.0, c1)
                c3 = lad("c3")
                nc.vector.tensor_sub(c3[:], u3[:], c1[:])

                sq2 = lad("sq")
                nc.scalar.square(sq2[:], c2[:])
                c4 = lad("c4")
                nc.vector.tensor_scalar(c4[:], sq2[:], 2.0, -1.0, ALU.mult, ALU.add)
                # exports for m=1..4
                nc.scalar.copy(pb(1)[:], c1[:])
                nc.scalar.copy(pb(2)[:], c2[:])
                nc.scalar.copy(pb(3)[:], c3[:])
                nc.vector.tensor_copy(pb(4)[:], c4[:])
                # m=5..6 straight to bf16 panel; m=7..8 to fp8 pair slots
                u5 = lad("u")
                stt(u5, c3, 2.0, c2)
                p5 = pb(5)
                nc.vector.tensor_sub(p5[:], u5[:], c1[:])
                sq3 = lad("sq")
                nc.scalar.square(sq3[:], c3[:])
                c6_dst = pfs("c6") if ic < FP6_ICS else pb(6)[:]
                nc.vector.tensor_scalar(
                    c6_dst, sq3[:], 2.0, -1.0, ALU.mult, ALU.add
                )
                u7 = lad("u")
                stt(u7, c4, 2.0, c3)
                nc.vector.tensor_sub(pfs("c7"), u7[:], c1[:])
                sq4 = lad("sq")
                nc.scalar.square(sq4[:], c4[:])
                nc.vector.tensor_scalar(
                    pfs("c8"), sq4[:], 2.0, -1.0, ALU.mult, ALU.add
                )

            # ---- mm emission ----------------------------------------
            # Per (ps, chunk) the 120 steps are emitted as 16 per-ic
            # tranches (2 per 2-ic block), consuming panel rows in
            # exactly generation order.  Tranche for even ic (block j):
            # [6 bf16, D1]; odd ic: [6 bf16, D2, D3].  (Batching the DR
            # steps instead measured slower overall: the per-DR stall
            # overlaps panel generation in the racing phase and the
            # reordering disturbed the DMA pipeline more than it saved.)

            def emit_bf16(ps, wgb, ic, pbt, first):
                k8o = (ic % 2) * NBF
                for n in range(5 if ic < FP6_ICS else NBF):
                    s = ic * NBF + n
                    nc.tensor.matmul(
                        ps[:], wgb[:, k8o + n, :], pbt[s][:],
                        start=(first and n == 0), stop=False,
                    )

            def dma_wgb(ot, j, nm=""):
                wgb = wpb.tile([128, 2 * NBF, 128], BF16, tag="wgb",
                               name=f"wgb{nm}")
                nc.sync.dma_start(wgb[:], wtb_ext[ot, j])
                return wgb

            def dma_wgf(ot, j, nm=""):
                wgf = wpf.tile([128, NDRX, 2, 128], FP8, tag="wgf",
                               name=f"wgf{nm}")
                nc.sync.dma_start(wgf[:], wtf_ext[ot, j])
                return wgf

            def emit_tranche(ps, t, ot, pbt, pft, boxes, nm=""):
                # All three DR steps of a block are emitted adjacently in
                # the odd-ic tranche: entering DoubleRow mode from a bf16
                # stream pays a ~190ns stall, so one entry per block
                # beats two (D1's inputs are ready even earlier).
                ic = t
                j = ic // 2
                if ic % 2 == 0:
                    boxes[0] = dma_wgb(ot, j, nm)
                    boxes[1] = dma_wgf(ot, j, nm)
                emit_bf16(ps, boxes[0], ic, pbt, first=(t == 0))
                wgf = boxes[1]
                if ic % 2 == 1:
                    ndr = NDRX if j < FP6_ICS // 2 else NDR
                    for d in range(ndr):
                        nc.tensor.matmul(
                            ps[:], wgf[:, d, :, :], pft[j * NDRX + d][:],
                            start=False, stop=(t == 15 and d == NDR - 1),
                            perf_mode=PM.DoubleRow,
                        )

            def gen_chunk(c):
                """Panel gen for chunk c (skipping pre-generated I-chunks).
                o-tile 0..4's matmul tranches are emitted interleaved so the
                TensorE does real GEMM work (and stays HAM-warm) while the
                panel is being generated."""
                pbt, pft, pre_ic = pre_ptiles.pop(c, ([None] * NSTEP_BF, {}, 0))
                pss = [
                    mmps.tile([128, tch], F32, tag="ps", name=f"ps{r}_{c}")
                    for r in range(n_race)
                ]
                boxes = [[None, None] for _ in range(n_race)]
                tr_next = 0

                def race_mm(tr_hi):
                    nonlocal tr_next
                    for t in range(tr_next, tr_hi):
                        for r in range(n_race):
                            emit_tranche(pss[r], t, r, pbt, pft, boxes[r],
                                         nm=f"r{r}_{c}_{t}")
                    tr_next = tr_hi

                for ic in range(nic):
                    if ic >= pre_ic:
                        # pair up transposes (8 back-to-back instead of 4)
                        # to halve the PE mode switches
                        if ic % 2 == 0 and ic + 1 < nic and (c, ic) not in tpm:
                            transpose_ic(c, ic)
                            transpose_ic(c, ic + 1)
                        gen_ic(c, ic, pbt, pft)
                    race_mm(ic + 1)
                race_mm(16)
                for r in range(n_race):
                    stg = stgp.tile([128, tch], F32, tag="stg",
                                    name=f"stg{r}_{c}")
                    nc.vector.tensor_copy(stg[:], pss[r][:])
                    nc.scalar.dma_start(
                        out_ext[ds(r * 128, 128), ds(c * tch, tch)], stg[:]
                    )
                return pbt, pft

            def mm_chunk(c, pbt, pft, nxt=None):
                if nxt is not None:
                    nxt_pbt = [None] * NSTEP_BF
                    nxt_pft = {}
                    pre_ptiles[nxt] = (nxt_pbt, nxt_pft, 7)
                for ot in range(n_race, n_ot):
                    ps = mmps.tile([128, tch], F32, tag="ps")
                    boxes = [None, None]
                    for t in range(16):
                        # pregen next chunk's first 6 I-chunks inside the
                        # tail of the last sweep (each ic's tags may only
                        # be overwritten once this sweep has consumed the
                        # current chunk's copy, i.e. after tranche ic).
                        if (nxt is not None and ot == n_ot - 1
                                and 2 <= t < 16 and t % 2 == 0):
                            ici = t // 2 - 1
                            if ici % 2 == 0:
                                transpose_ic(nxt, ici)
                                transpose_ic(nxt, ici + 1)
                            gen_ic(nxt, ici, nxt_pbt, nxt_pft)
                        emit_tranche(ps, t, ot, pbt, pft, boxes)
                    stg = stgp.tile([128, tch], F32, tag="stg")
                    nc.vector.tensor_copy(stg[:], ps[:])
                    nc.scalar.dma_start(
                        out_ext[ds(ot * 128, 128), ds(c * tch, tch)], stg[:]
                    )

            preamble(0)
            for c in range(nch):
                pbt, pft = gen_chunk(c)
                if c + 1 < nch:
                    preamble(c + 1)
                mm_chunk(c, pbt, pft, nxt=(c + 1) if c + 1 < nch else None)

    _optimize_sems(nc)
    nc.compile()
    return nc


def _optimize_sems(nc):
    """Post-schedule IR pass: engine instructions complete in queue order, so
    a monotone per-engine counter semaphore only needs an increment at the
    positions some wait actually references.  Strip the rest and renumber the
    wait thresholds.  Also drop waits dominated by an earlier wait on the
    same engine queue.  Semaphores touched by DMA completions or any
    non-inc update are left alone."""
    ENG_FIFO = {
        mybir.EngineType.PE,
        mybir.EngineType.Activation,
        mybir.EngineType.DVE,
        mybir.EngineType.Pool,
        mybir.EngineType.SP,
    }
    f = nc.m.functions[0]
    insts = [i for bb in f.blocks for i in bb.instructions]

    upd_insts = {}   # sem id -> list of (inst, engine, value) in program order
    upd_ok = {}      # sem id -> eligible for stripping
    waited = {}      # sem id -> set of imm values referenced
    wait_bad = set()  # sems with register/non-ge waits
    for inst in insts:
        si = inst.sync_info
        if not si:
            continue
        is_dma = "DMA" in type(inst).__name__ or "Dma" in type(inst).__name__
        for u in (si.on_update or []):
            upd_insts.setdefault(u.id, []).append((inst, u))
            ok = upd_ok.get(u.id, True)
            if (is_dma or inst.engine not in ENG_FIFO
                    or u.update_mode != "sem-inc" or u.update_value != 1
                    or u.update_reg is not None):
                ok = False
            if any(e != inst.engine for (pi, pu) in upd_insts[u.id] for e in [pi.engine]):
                ok = False
            upd_ok[u.id] = ok
        for w in (si.on_wait or []):
            if w.wait_reg is not None or w.wait_mode != "sem-ge-imm":
                wait_bad.add(w.id)
            else:
                waited.setdefault(w.id, set()).add(w.wait_value)

    # monotone sems: every update is a positive immediate inc/add (wait-ge on
    # these can never be un-satisfied, so dominated waits are droppable)
    monotone = set()
    for sid, lst in upd_insts.items():
        if all(u.update_mode in ("sem-inc", "sem-add-imm")
               and u.update_reg is None and (u.update_value or 0) > 0
               for (_, u) in lst):
            monotone.add(sid)

    remap = {}  # sem id -> {old_val: new_val}
    keep_pos = {}  # sem id -> set of cumulative counts to keep
    for sid, lst in upd_insts.items():
        if not upd_ok.get(sid) or sid in wait_bad:
            continue
        total = len(lst)
        refs = sorted(v for v in waited.get(sid, ()) if 1 <= v <= total)
        if any(v > total or v < 1 for v in waited.get(sid, ())):
            continue
        if total not in refs:
            refs.append(total)  # keep the final count reachable for drains
        remap[sid] = {v: i + 1 for i, v in enumerate(refs)}
        keep_pos[sid] = set(refs)

    n_strip = n_keep = n_wdrop = 0
    counts = {sid: 0 for sid in remap}
    eng_wait_max = {}  # (engine, sem) -> max value already waited on that queue
    for inst in insts:
        si = inst.sync_info
        if not si:
            continue
        new_upd, new_wait, changed = [], [], False
        for u in (si.on_update or []):
            if u.id in remap:
                counts[u.id] += 1
                if counts[u.id] in keep_pos[u.id]:
                    new_upd.append(u)
                    n_keep += 1
                else:
                    changed = True
                    n_strip += 1
            else:
                new_upd.append(u)
        for w in (si.on_wait or []):
            v = w.wait_value
            if w.id in remap and w.wait_reg is None and w.wait_mode == "sem-ge-imm":
                v = remap[w.id][w.wait_value]
            key = (inst.engine, w.id)
            is_imm = w.wait_reg is None and w.wait_mode == "sem-ge-imm"
            if is_imm and w.id in monotone and eng_wait_max.get(key, 0) >= v:
                changed = True
                n_wdrop += 1
                continue
            if is_imm and w.id in monotone:
                eng_wait_max[key] = max(eng_wait_max.get(key, 0), v)
            if v != w.wait_value:
                w = mybir.SyncWait(sync_type=w.sync_type, id=w.id,
                                   ant_name=w.ant_name, wait_mode=w.wait_mode,
                                   wait_value=v, wait_reg=None)
                changed = True
            new_wait.append(w)
        if changed:
            inst.sync_info = mybir.SyncInfo(on_wait=new_wait, on_update=new_upd)
    print(f"_optimize_sems: stripped {n_strip} incs (kept {n_keep}), "
          f"dropped {n_wdrop} dominated waits")


def prep_weights(base_weight, spline_weight, nO=O, nI=I):
    """Host-side: build the bf16 and fp8 weight planes in the kernel's
    k-step order, pre-tiled for contiguous DMAs.

    Returns dict with:
      wtb [n_ot, nblk, 128, 12, 128] bf16 — rows cos1..6 of ics (2j, 2j+1)
      wtf [n_ot, nblk, 128, 3, 2, 128] fp8 — block j's DR pairs
          D1=(silu_e,c7_e), D2=(c8_e,silu_o), D3=(c7_o,c8_o)
    """
    nic = nI // 128
    nblk = nic // 2
    n_ot = nO // 128
    bwT = base_weight.T.astype(np.float32)                 # [i, o]
    swT = spline_weight.transpose(1, 2, 0).astype(np.float32)  # [i, g, o]

    def blk(arr_io, ic, ot):
        # [128, 128] slice: rows i in ic, cols o in ot
        return arr_io[ic * 128:(ic + 1) * 128, ot * 128:(ot + 1) * 128]

    wtb = np.empty((n_ot, nblk, 128, 2 * NBF, 128), np.float32)
    wtf = np.empty((n_ot, nblk, 128, NDRX, 2, 128), np.float32)
    for ot in range(n_ot):
        for j in range(nblk):
            e, o_ = 2 * j, 2 * j + 1
            for m in range(NBF):          # cos1..cos6
                wtb[ot, j, :, m, :] = blk(swT[:, m, :], e, ot)
                wtb[ot, j, :, NBF + m, :] = blk(swT[:, m, :], o_, ot)
            wtf[ot, j, :, 0, 0, :] = blk(bwT, e, ot)           # silu_e
            wtf[ot, j, :, 0, 1, :] = blk(swT[:, 6, :], e, ot)  # c7_e
            wtf[ot, j, :, 1, 0, :] = blk(swT[:, 7, :], e, ot)  # c8_e
            wtf[ot, j, :, 1, 1, :] = blk(bwT, o_, ot)          # silu_o
            wtf[ot, j, :, 2, 0, :] = blk(swT[:, 6, :], o_, ot)  # c7_o
            wtf[ot, j, :, 2, 1, :] = blk(swT[:, 7, :], o_, ot)  # c8_o
            wtf[ot, j, :, 3, 0, :] = blk(swT[:, 5, :], e, ot)   # c6_e
            wtf[ot, j, :, 3, 1, :] = blk(swT[:, 5, :], o_, ot)  # c6_o
    return {
        "wtb": np.ascontiguousarray(wtb.astype(ml_dtypes.bfloat16)),
        "wtf": np.ascontiguousarray(wtf.astype(ml_dtypes.float8_e4m3fn)),
    }


_NC_CACHE = {}


def _get_nc():
    if "nc" not in _NC_CACHE:
        _NC_CACHE["nc"] = build_nc()
    return _NC_CACHE["nc"]


def kernel(x, ln_weight, ln_bias, base_weight, spline_weight):
    x = np.asarray(x, np.float32)
    ln_weight = np.asarray(ln_weight, np.float32)
    ln_bias = np.asarray(ln_bias, np.float32)
    wts = prep_weights(np.asarray(base_weight, np.float32),
                       np.asarray(spline_weight, np.float32))
    nc = _get_nc()
    in_maps = [
        {
            "x": np.ascontiguousarray(x[b]),
            "lnw": ln_weight,
            "lnb": ln_bias,
            "wtb": wts["wtb"],
            "wtf": wts["wtf"],
        }
        for b in range(B)
    ]
    res = run_bass_kernel_spmd(nc, in_maps, core_ids=list(range(B)))
    out = np.stack([res.results[b]["out"].T for b in range(B)])
    return np.ascontiguousarray(out.astype(np.float32))


# revision 40
# speedup vs baseline: 1.0114x; 1.0114x over previous
"""Trainium2 Bass kernel for AdvancedKANLayer.

Math (per reference):
  xn    = LayerNorm(x) * ln_w + ln_b           (eps=1e-5)
  base  = silu(xn) @ base_weight.T             [B,S,O]
  t     = tanh(xn)
  basis = cos(pi*k*t), k=1..8
  spl   = einsum('bsig,oig->bso', basis, spline_weight)
  out   = base + spl
Strategy: data-parallel over batch (8 cores, one batch entry each, no
collectives).  Per core the whole thing is one K=18432 GEMM:
  out[o, t] = sum_k W_all[k, o] * panel[k, t]
where panel rows are [silu(xn); cos(1*pi*t); ...; cos(8*pi*t)] per
I-chunk, generated on-chip.  cos(k*pi*t) is built from
c1 = cos(pi*t) = 1 - 2*sin(pi*t/2)^2 via Chebyshev product
identities on the VectorEngine (ScalarE Sin is only valid on [-pi,pi]).

Mixed precision (rel-err budget 2e-2, measured ~1.7e-2): rows
{silu, cos7, cos8} run as fp8-e4m3 and are packed two-per-step into
MatmulPerfMode.DoubleRow matmuls (HW-verified: a DoubleRow pair-step
contracting 256 k costs the same ~216 ns as one bf16 128-k step, i.e.
2x throughput); rows cos1..cos6 stay bf16.  Per I-chunk that is
6 bf16 steps + 1.5 DoubleRow steps = 7.5 step-equivalents instead of
9 -> ~1.2x kernel speedup.  fp8 rows are paired per 2-ic block:
D1=(silu_e, c7_e), D2=(c8_e, silu_o), D3=(c7_o, c8_o), so all pair
tiles fill in generation order.  Weights are pre-transposed/pre-tiled
on the host (bf16 + fp8 planes); f32 PSUM accumulation throughout.

Perf notes (measured on HW): the N=512 matmul stream floor is ~216
ns/MM (1 col/cycle @2.4GHz) and LDWEIGHTS hides fully as long as the
weight DMAs stay ahead.  To that end: output DMAs issue on the ACT
HWDGE queue so the SP queue only carries weight/x DMAs; weight DMAs
move one 2-ic block per transfer (12 bf16 steps + 3 DR steps as two
DMAs) with deep pools; 5 o-tiles race the panel generation; the next
chunk's first six I-chunks are generated inside the tail of the
current mm sweep so the PE crosses chunk boundaries without idling;
240 warmup matmuls keep the HAM clock-gate open during the initial
LayerNorm; I-chunk transposes are emitted in pairs (8 back-to-back)
to halve PE mode switches; and a post-schedule pass (_optimize_sems)
strips unreferenced semaphore increments.
"""

import math
import sys
import types

try:  # some images lack antenv.axon_hooks, which bass_utils imports
    import antenv.axon_hooks  # noqa: F401
except Exception:
    try:
        import antenv
        _hooks = {}
        _m = types.ModuleType("antenv.axon_hooks")
        _m.set_axon_ntff_profile_hook = lambda h: _hooks.__setitem__("h", h)
        _m.get_axon_ntff_profile_hook = lambda: _hooks.get("h")
        sys.modules["antenv.axon_hooks"] = _m
        antenv.axon_hooks = _m
    except Exception:
        pass

import numpy as np
import ml_dtypes

import concourse.bass as bass
import concourse.mybir as mybir
import concourse.tile as tile
from concourse import bacc
from concourse import masks
from concourse.bass import ds, ts
from concourse.bass_utils import run_bass_kernel_spmd

F32 = mybir.dt.float32
BF16 = mybir.dt.bfloat16
FP8 = mybir.dt.float8e4
AF = mybir.ActivationFunctionType
ALU = mybir.AluOpType
PM = mybir.MatmulPerfMode

EPS = 1e-5

# geometry (full problem, per core)
B = 8
T = 2048          # tokens per core (= S, one batch entry per core)
I = 2048          # input dim
O = 2048          # output dim
G = 8             # cos harmonics
TCH = 512         # token chunk (matmul N)
NCH = T // TCH    # 4
NIC = I // 128    # 16 I-chunks
NBLK = NIC // 2   # 8 2-ic blocks
NBF = 6           # bf16 rows per ic (cos1..cos6)
NDR = 3           # DoubleRow pair-steps per regular 2-ic block
NDRX = 4          # max DR pair-steps (blocks 0-1 carry a 4th: c6 pair)
FP6_ICS = 4       # ics 0..3 run cos6 in fp8 too (quarter-row err spend)
NOT = O // 128    # 16 o-tiles
# per-chunk step counts
NSTEP_BF = NIC * NBF        # 96
NSTEP_DR = NBLK * NDR       # 24
NSTEP = NSTEP_BF + NSTEP_DR  # 120 PE matmul instructions per (ot, chunk)


def build_nc(nT=T, nI=I, nO=O, tch=TCH):
    nch = nT // tch
    nic = nI // 128
    nblk = nic // 2
    n_ot = nO // 128
    ntt = tch // 128          # token-tiles per chunk

    n_race = min(5, n_ot - 1) if n_ot > 1 else 1

    nc = bacc.Bacc("TRN2", target_bir_lowering=False, debug=False)
    x_ext = nc.declare_dram_parameter("x", [nT, nI], F32, isOutput=False)
    lnw_ext = nc.declare_dram_parameter("lnw", [nI], F32, isOutput=False)
    lnb_ext = nc.declare_dram_parameter("lnb", [nI], F32, isOutput=False)
    wtb_ext = nc.declare_dram_parameter(
        "wtb", [n_ot, nblk, 128, 2 * NBF, 128], BF16, isOutput=False)
    wtf_ext = nc.declare_dram_parameter(
        "wtf", [n_ot, nblk, 128, NDRX, 2, 128], FP8, isOutput=False)
    out_ext = nc.declare_dram_parameter("out", [nO, nT], F32, isOutput=True)

    with tile.TileContext(nc) as tc:
        with (
            tc.tile_pool(name="consts", bufs=1) as consts,
            tc.tile_pool(name="xp", bufs=4) as xpool,
            tc.tile_pool(name="statp", bufs=2) as statp,
            tc.tile_pool(name="genp", bufs=1) as genp,
            tc.tile_pool(name="ladp", bufs=1) as ladp,
            tc.tile_pool(name="panelp", bufs=1) as panelp,
            tc.tile_pool(name="wpb", bufs=6) as wpb,
            tc.tile_pool(name="wpf", bufs=6) as wpf,
            tc.tile_pool(name="stgp", bufs=2) as stgp,
            tc.tile_pool(name="tpps", bufs=3, space="PSUM") as tpps,
            tc.tile_pool(name="mmps", bufs=5, space="PSUM") as mmps,
        ):
            identity = consts.tile([128, 128], F32)
            masks.make_identity(nc, identity[:])
            lnw_sb = consts.tile([128, nic], F32)
            nc.sync.dma_start(lnw_sb[:], lnw_ext.rearrange("(f p) -> p f", p=128))
            lnb_sb = consts.tile([128, nic], F32)
            nc.sync.dma_start(lnb_sb[:], lnb_ext.rearrange("(f p) -> p f", p=128))
            eps_sb = consts.tile([128, 1], F32)
            nc.vector.memset(eps_sb[:], EPS)
            zb = consts.tile([128, 128], BF16)
            nc.vector.memset(zb[:], 0.0)

            # PE warmup: keep HAM busy while the first chunk's LN runs so
            # the first real matmuls start at full clock.
            wps = mmps.tile([128, tch], F32, tag="ps", name="warm_ps")
            for _ in range(320):
                nc.tensor.matmul(wps[:, 0:128], zb[:], zb[:])

            state = {}
            tpm = {}

            def preamble(c):
                """x DMA + LN stats + in-place normalize for chunk c.
                Stats/normalize are per token-tile so the first tile is
                ready after one x DMA, not four."""
                xnts = []
                for j in range(ntt):
                    xt = xpool.tile([128, nI], F32, tag="xt")
                    nc.sync.dma_start(xt[:], x_ext[ds((c * ntt + j) * 128, 128), :])
                    bn6 = statp.tile([128, 4, 6], F32, tag="bn6")
                    for q in range(4):
                        nc.vector.bn_stats(
                            bn6[:, q, :], xt[:, ds(q * (nI // 4), nI // 4)]
                        )
                    stats = statp.tile([128, 2], F32, tag="stats")
                    nc.vector.bn_aggr(stats[:], bn6[:])
                    std = statp.tile([128, 1], F32, tag="std")
                    nc.scalar.activation(
                        std[:], stats[:, 1:2], AF.Sqrt, bias=eps_sb[:]
                    )
                    istd = statp.tile([128, 1], F32, tag="istd")
                    nc.vector.reciprocal(istd[:], std[:])
                    nmi = statp.tile([128, 1], F32, tag="nmi")
                    nc.vector.scalar_tensor_tensor(
                        nmi[:], stats[:, 0:1], -1.0, istd[:], ALU.mult, ALU.mult
                    )
                    # normalize in place: xn = (x - mu) * istd
                    nc.scalar.activation(
                        xt[:], xt[:], AF.Identity, bias=nmi[:], scale=istd[:],
                    )
                    xnts.append(xt)
                state[c] = xnts

            def transpose_ic(c, ic):
                """PE-transpose I-chunk ic of chunk c into a PSUM tile."""
                xnts = state[c]
                tp = tpps.tile([128, tch], F32, tag="tp", name=f"tp_{c}_{ic}")
                for j in range(ntt):
                    nc.tensor.transpose(
                        tp[:, ts(j, 128)], xnts[j][:, ts(ic, 128)], identity[:]
                    )
                tpm[(c, ic)] = tp
                return tp

            pre_ptiles = {}

            def gen_ic(c, ic, pbt, pft):
                """Transpose + tanh/silu + cheb ladder for I-chunk ic of
                chunk c.  bf16 rows cos1..cos6 fill pbt[ic*6 .. ic*6+5];
                fp8 rows (silu, cos7, cos8) fill their pair-tile slots in
                pft (3 pair tiles per 2-ic block)."""
                tp = tpm.pop((c, ic), None)
                if tp is None:
                    tp = transpose_ic(c, ic)
                lw = lnw_sb[:, ic : ic + 1]
                lb = lnb_sb[:, ic : ic + 1]
                j = ic // 2
                odd = ic % 2

                def pb(m):
                    # bf16 panel tile for cos_m (m=1..6)
                    s = ic * NBF + (m - 1)
                    t_ = panelp.tile(
                        [128, tch], BF16, tag=f"pb{s:03d}", name=f"pb_{c}_{s:03d}"
                    )
                    pbt[s] = t_
                    return t_

                def pfs(which):
                    # fp8 pair-tile slot for silu/cos7/cos8 (+cos6 on
                    # ics < FP6_ICS) of this ic
                    if not odd:
                        d, slot = {"silu": (0, 0), "c7": (0, 1), "c8": (1, 0),
                                   "c6": (3, 0)}[which]
                    else:
                        d, slot = {"silu": (1, 1), "c7": (2, 0), "c8": (2, 1),
                                   "c6": (3, 1)}[which]
                    di = j * NDRX + d
                    t_ = pft.get(di)
                    if t_ is None:
                        t_ = panelp.tile(
                            [128, 2, tch], FP8, tag=f"pf{di:02d}",
                            name=f"pf_{c}_{di:02d}"
                        )
                        pft[di] = t_
                    return t_[:, slot, :]

                th = genp.tile([128, tch], F32, tag="th")
                nc.scalar.activation(th[:], tp[:], AF.Tanh, bias=lb, scale=lw)

                nc.scalar.activation(pfs("silu"), tp[:], AF.Silu, bias=lb, scale=lw)
                sh = genp.tile([128, tch], F32, tag="sh")
                nc.scalar.activation(sh[:], th[:], AF.Sin, scale=math.pi / 2)

                def lad(tag):
                    return ladp.tile(
                        [128, tch], F32, tag=tag, name=f"lad_{tag}_{c}_{ic}"
                    )

                def stt(out, a, s, b):
                    nc.vector.scalar_tensor_tensor(
                        out[:], a[:], s, b[:], ALU.mult, ALU.mult
                    )

                # c1 = 1 - 2*sh^2
                u = lad("u")
                stt(u, sh, -2.0, sh)
                c1 = lad("c1")
                nc.vector.tensor_scalar_add(c1[:], u[:], 1.0)
                # squares on ScalarE to offload DVE
                sq1 = lad("sq")
                nc.scalar.square(sq1[:], c1[:])
                c2 = lad("c2")
                nc.vector.tensor_scalar(c2[:], sq1[:], 2.0, -1.0, ALU.mult, ALU.add)
                # c3 = 2*c1*c2 - c1
                u3 = lad("u")
                stt(u3, c2, 2.0, c1)
                c3 = lad("c3")
                nc.vector.tensor_sub(c3[:], u3[:], c1[:])

                sq2 = lad("sq")
                nc.scalar.square(sq2[:], c2[:])
                c4 = lad("c4")
                nc.vector.tensor_scalar(c4[:], sq2[:], 2.0, -1.0, ALU.mult, ALU.add)
                # exports for m=1..4
                nc.scalar.copy(pb(1)[:], c1[:])
                nc.scalar.copy(pb(2)[:], c2[:])
                nc.scalar.copy(pb(3)[:], c3[:])
                nc.vector.tensor_copy(pb(4)[:], c4[:])
                # m=5..6 straight to bf16 panel; m=7..8 to fp8 pair slots
                u5 = lad("u")
                stt(u5, c3, 2.0, c2)
                p5 = pb(5)
                nc.vector.tensor_sub(p5[:], u5[:], c1[:])
                sq3 = lad("sq")
                nc.scalar.square(sq3[:], c3[:])
                c6_dst = pfs("c6") if ic < FP6_ICS else pb(6)[:]
                nc.vector.tensor_scalar(
                    c6_dst, sq3[:], 2.0, -1.0, ALU.mult, ALU.add
                )
                u7 = lad("u")
                stt(u7, c4, 2.0, c3)
                nc.vector.tensor_sub(pfs("c7"), u7[:], c1[:])
                sq4 = lad("sq")
                nc.scalar.square(sq4[:], c4[:])
                nc.vector.tensor_scalar(
                    pfs("c8"), sq4[:], 2.0, -1.0, ALU.mult, ALU.add
                )

            # ---- mm emission ----------------------------------------
            # Per (ps, chunk) the 120 steps are emitted as 16 per-ic
            # tranches (2 per 2-ic block), consuming panel rows in
            # exactly generation order.  Tranche for even ic (block j):
            # [6 bf16, D1]; odd ic: [6 bf16, D2, D3].  (Batching the DR
            # steps instead measured slower overall: the per-DR stall
            # overlaps panel generation in the racing phase and the
            # reordering disturbed the DMA pipeline more than it saved.)

            def emit_bf16(ps, wgb, ic, pbt, first):
                k8o = (ic % 2) * NBF
                for n in range(5 if ic < FP6_ICS else NBF):
                    s = ic * NBF + n
                    nc.tensor.matmul(
                        ps[:], wgb[:, k8o + n, :], pbt[s][:],
                        start=(first and n == 0), stop=False,
                    )

            def dma_wgb(ot, j, nm=""):
                wgb = wpb.tile([128, 2 * NBF, 128], BF16, tag="wgb",
                               name=f"wgb{nm}")
                nc.sync.dma_start(wgb[:], wtb_ext[ot, j])
                return wgb

            def dma_wgf(ot, j, nm=""):
                wgf = wpf.tile([128, NDRX, 2, 128], FP8, tag="wgf",
                               name=f"wgf{nm}")
                nc.sync.dma_start(wgf[:], wtf_ext[ot, j])
                return wgf

            def emit_tranche(ps, t, ot, pbt, pft, boxes, nm=""):
                # All three DR steps of a block are emitted adjacently in
                # the odd-ic tranche: entering DoubleRow mode from a bf16
                # stream pays a ~190ns stall, so one entry per block
                # beats two (D1's inputs are ready even earlier).
                ic = t
                j = ic // 2
                if ic % 2 == 0:
                    boxes[0] = dma_wgb(ot, j, nm)
                    boxes[1] = dma_wgf(ot, j, nm)
                emit_bf16(ps, boxes[0], ic, pbt, first=(t == 0))
                wgf = boxes[1]
                if ic % 2 == 1:
                    ndr = NDRX if j < FP6_ICS // 2 else NDR
                    for d in range(ndr):
                        nc.tensor.matmul(
                            ps[:], wgf[:, d, :, :], pft[j * NDRX + d][:],
                            start=False, stop=(t == 15 and d == NDR - 1),
                            perf_mode=PM.DoubleRow,
                        )

            def gen_chunk(c):
                """Panel gen for chunk c (skipping pre-generated I-chunks).
                o-tile 0..4's matmul tranches are emitted interleaved so the
                TensorE does real GEMM work (and stays HAM-warm) while the
                panel is being generated."""
                pbt, pft, pre_ic = pre_ptiles.pop(c, ([None] * NSTEP_BF, {}, 0))
                pss = [
                    mmps.tile([128, tch], F32, tag="ps", name=f"ps{r}_{c}")
                    for r in range(n_race)
                ]
                boxes = [[None, None] for _ in range(n_race)]
                tr_next = 0

                def race_mm(tr_hi):
                    nonlocal tr_next
                    for t in range(tr_next, tr_hi):
                        for r in range(n_race):
                            emit_tranche(pss[r], t, r, pbt, pft, boxes[r],
                                         nm=f"r{r}_{c}_{t}")
                    tr_next = tr_hi

                for ic in range(nic):
                    if ic >= pre_ic:
                        # pair up transposes (8 back-to-back instead of 4)
                        # to halve the PE mode switches
                        if ic % 2 == 0 and ic + 1 < nic and (c, ic) not in tpm:
                            transpose_ic(c, ic)
                            transpose_ic(c, ic + 1)
                        gen_ic(c, ic, pbt, pft)
                    race_mm(ic + 1)
                race_mm(16)
                for r in range(n_race):
                    stg = stgp.tile([128, tch], F32, tag="stg",
                                    name=f"stg{r}_{c}")
                    nc.vector.tensor_copy(stg[:], pss[r][:])
                    nc.scalar.dma_start(
                        out_ext[ds(r * 128, 128), ds(c * tch, tch)], stg[:]
                    )
                return pbt, pft

            def mm_chunk(c, pbt, pft, nxt=None):
                if nxt is not None:
                    nxt_pbt = [None] * NSTEP_BF
                    nxt_pft = {}
                    pre_ptiles[nxt] = (nxt_pbt, nxt_pft, 7)
                for ot in range(n_race, n_ot):
                    ps = mmps.tile([128, tch], F32, tag="ps")
                    boxes = [None, None]
                    for t in range(16):
                        # pregen next chunk's first 6 I-chunks inside the
                        # tail of the last sweep (each ic's tags may only
                        # be overwritten once this sweep has consumed the
                        # current chunk's copy, i.e. after tranche ic).
                        if (nxt is not None and ot == n_ot - 1
                                and 2 <= t < 16 and t % 2 == 0):
                            ici = t // 2 - 1
                            if ici % 2 == 0:
                                transpose_ic(nxt, ici)
                                transpose_ic(nxt, ici + 1)
                            gen_ic(nxt, ici, nxt_pbt, nxt_pft)
                        emit_tranche(ps, t, ot, pbt, pft, boxes)
                    stg = stgp.tile([128, tch], F32, tag="stg")
                    nc.vector.tensor_copy(stg[:], ps[:])
                    nc.scalar.dma_start(
                        out_ext[ds(ot * 128, 128), ds(c * tch, tch)], stg[:]
                    )

            preamble(0)
            for c in range(nch):
                pbt, pft = gen_chunk(c)
                if c + 1 < nch:
                    preamble(c + 1)
                mm_chunk(c, pbt, pft, nxt=(c + 1) if c + 1 < nch else None)

    _optimize_sems(nc)
    nc.compile()
    return nc


def _optimize_sems(nc):
    """Post-schedule IR pass: engine instructions complete in queue order, so
    a monotone per-engine counter semaphore only needs an increment at the
    positions some wait actually references.  Strip the rest and renumber the
    wait thresholds.  Also drop waits dominated by an earlier wait on the
    same engine queue.  Semaphores touched by DMA completions or any
    non-inc update are left alone."""
    ENG_FIFO = {
        mybir.EngineType.PE,
        mybir.EngineType.Activation,
        mybir.EngineType.DVE,
        mybir.EngineType.Pool,
        mybir.EngineType.SP,
    }
    f = nc.m.functions[0]
    insts = [i for bb in f.blocks for i in bb.instructions]

    upd_insts = {}   # sem id -> list of (inst, engine, value) in program order
    upd_ok = {}      # sem id -> eligible for stripping
    waited = {}      # sem id -> set of imm values referenced
    wait_bad = set()  # sems with register/non-ge waits
    for inst in insts:
        si = inst.sync_info
        if not si:
            continue
        is_dma = "DMA" in type(inst).__name__ or "Dma" in type(inst).__name__
        for u in (si.on_update or []):
            upd_insts.setdefault(u.id, []).append((inst, u))
            ok = upd_ok.get(u.id, True)
            if (is_dma or inst.engine not in ENG_FIFO
                    or u.update_mode != "sem-inc" or u.update_value != 1
                    or u.update_reg is not None):
                ok = False
            if any(e != inst.engine for (pi, pu) in upd_insts[u.id] for e in [pi.engine]):
                ok = False
            upd_ok[u.id] = ok
        for w in (si.on_wait or []):
            if w.wait_reg is not None or w.wait_mode != "sem-ge-imm":
                wait_bad.add(w.id)
            else:
                waited.setdefault(w.id, set()).add(w.wait_value)

    # monotone sems: every update is a positive immediate inc/add (wait-ge on
    # these can never be un-satisfied, so dominated waits are droppable)
    monotone = set()
    for sid, lst in upd_insts.items():
        if all(u.update_mode in ("sem-inc", "sem-add-imm")
               and u.update_reg is None and (u.update_value or 0) > 0
               for (_, u) in lst):
            monotone.add(sid)

    remap = {}  # sem id -> {old_val: new_val}
    keep_pos = {}  # sem id -> set of cumulative counts to keep
    for sid, lst in upd_insts.items():
        if not upd_ok.get(sid) or sid in wait_bad:
            continue
        total = len(lst)
        refs = sorted(v for v in waited.get(sid, ()) if 1 <= v <= total)
        if any(v > total or v < 1 for v in waited.get(sid, ())):
            continue
        if total not in refs:
            refs.append(total)  # keep the final count reachable for drains
        remap[sid] = {v: i + 1 for i, v in enumerate(refs)}
        keep_pos[sid] = set(refs)

    n_strip = n_keep = n_wdrop = 0
    counts = {sid: 0 for sid in remap}
    eng_wait_max = {}  # (engine, sem) -> max value already waited on that queue
    for inst in insts:
        si = inst.sync_info
        if not si:
            continue
        new_upd, new_wait, changed = [], [], False
        for u in (si.on_update or []):
            if u.id in remap:
                counts[u.id] += 1
                if counts[u.id] in keep_pos[u.id]:
                    new_upd.append(u)
                    n_keep += 1
                else:
                    changed = True
                    n_strip += 1
            else:
                new_upd.append(u)
        for w in (si.on_wait or []):
            v = w.wait_value
            if w.id in remap and w.wait_reg is None and w.wait_mode == "sem-ge-imm":
                v = remap[w.id][w.wait_value]
            key = (inst.engine, w.id)
            is_imm = w.wait_reg is None and w.wait_mode == "sem-ge-imm"
            if is_imm and w.id in monotone and eng_wait_max.get(key, 0) >= v:
                changed = True
                n_wdrop += 1
                continue
            if is_imm and w.id in monotone:
                eng_wait_max[key] = max(eng_wait_max.get(key, 0), v)
            if v != w.wait_value:
                w = mybir.SyncWait(sync_type=w.sync_type, id=w.id,
                                   ant_name=w.ant_name, wait_mode=w.wait_mode,
                                   wait_value=v, wait_reg=None)
                changed = True
            new_wait.append(w)
        if changed:
            inst.sync_info = mybir.SyncInfo(on_wait=new_wait, on_update=new_upd)
    print(f"_optimize_sems: stripped {n_strip} incs (kept {n_keep}), "
          f"dropped {n_wdrop} dominated waits")


def prep_weights(base_weight, spline_weight, nO=O, nI=I):
    """Host-side: build the bf16 and fp8 weight planes in the kernel's
    k-step order, pre-tiled for contiguous DMAs.

    Returns dict with:
      wtb [n_ot, nblk, 128, 12, 128] bf16 — rows cos1..6 of ics (2j, 2j+1)
      wtf [n_ot, nblk, 128, 3, 2, 128] fp8 — block j's DR pairs
          D1=(silu_e,c7_e), D2=(c8_e,silu_o), D3=(c7_o,c8_o)
    """
    nic = nI // 128
    nblk = nic // 2
    n_ot = nO // 128
    bwT = base_weight.T.astype(np.float32)                 # [i, o]
    swT = spline_weight.transpose(1, 2, 0).astype(np.float32)  # [i, g, o]

    def blk(arr_io, ic, ot):
        # [128, 128] slice: rows i in ic, cols o in ot
        return arr_io[ic * 128:(ic + 1) * 128, ot * 128:(ot + 1) * 128]

    wtb = np.empty((n_ot, nblk, 128, 2 * NBF, 128), np.float32)
    wtf = np.empty((n_ot, nblk, 128, NDRX, 2, 128), np.float32)
    for ot in range(n_ot):
        for j in range(nblk):
            e, o_ = 2 * j, 2 * j + 1
            for m in range(NBF):          # cos1..cos6
                wtb[ot, j, :, m, :] = blk(swT[:, m, :], e, ot)
                wtb[ot, j, :, NBF + m, :] = blk(swT[:, m, :], o_, ot)
            wtf[ot, j, :, 0, 0, :] = blk(bwT, e, ot)           # silu_e
            wtf[ot, j, :, 0, 1, :] = blk(swT[:, 6, :], e, ot)  # c7_e
            wtf[ot, j, :, 1, 0, :] = blk(swT[:, 7, :], e, ot)  # c8_e
            wtf[ot, j, :, 1, 1, :] = blk(bwT, o_, ot)          # silu_o
            wtf[ot, j, :, 2, 0, :] = blk(swT[:, 6, :], o_, ot)  # c7_o
            wtf[ot, j, :, 2, 1, :] = blk(swT[:, 7, :], o_, ot)  # c8_o
            wtf[ot, j, :, 3, 0, :] = blk(swT[:, 5, :], e, ot)   # c6_e
            wtf[ot, j, :, 3, 1, :] = blk(swT[:, 5, :], o_, ot)  # c6_o
    return {
        "wtb": np.ascontiguousarray(wtb.astype(ml_dtypes.bfloat16)),
        "wtf": np.ascontiguousarray(wtf.astype(ml_dtypes.float8_e4m3fn)),
    }


_NC_CACHE = {}


def _get_nc():
    if "nc" not in _NC_CACHE:
        _NC_CACHE["nc"] = build_nc()
    return _NC_CACHE["nc"]


def kernel(x, ln_weight, ln_bias, base_weight, spline_weight):
    x = np.asarray(x, np.float32)
    ln_weight = np.asarray(ln_weight, np.float32)
    ln_bias = np.asarray(ln_bias, np.float32)
    wts = prep_weights(np.asarray(base_weight, np.float32),
                       np.asarray(spline_weight, np.float32))
    nc = _get_nc()
    in_maps = [
        {
            "x": np.ascontiguousarray(x[b]),
            "lnw": ln_weight,
            "lnb": ln_bias,
            "wtb": wts["wtb"],
            "wtf": wts["wtf"],
        }
        for b in range(B)
    ]
    res = run_bass_kernel_spmd(nc, in_maps, core_ids=list(range(B)))
    out = np.stack([res.results[b]["out"].T for b in range(B)])
    return np.ascontiguousarray(out.astype(np.float32))
